# revision 12
# baseline (speedup 1.0000x reference)
"""Trainium2 Bass kernel for nn_Block_40175124087290 (segment_reduce).

Computation (see reference): three dilated 5x5 convs (55->6 ch, dil 1/2/3) ->
feat [B,18,H,W]; two 1x1 convs (top 18->36, bot 18->1) are only needed through
their grid-cell means, which reduce (by linearity) to channel-weighted cell
sums of feat. Output channels: 0-35 broadcast top-mean map, 36-53 feat,
54 sigmoid(bot-mean map); plus pools = sigmoid(bot means) [B,1,30,20].

Sharding: 8 cores = (batch 4) x (H halves 2). Per-cell raw sums are
all-reduced across cores (tiny [30,40] f32 collective).

Conv mapping: per (conv, row-pair) 25 tap-matmuls [K=110 block-diag(2 rows),
M=12, N=512] in float32r (1 cyc/col), kh/kw shifts as rhs AP offsets into a
zero-padded SBUF x tile, PSUM accumulated, 4-way col-group packed via
tile_position. ACT epilogue applies bias+relu into SBUF feat tiles.

Cell stats: feat is re-read from the output DRAM tensor per channel as
[128 rows, 512] tiles (rows land on partitions), contracted with host-built
(one-hot x channel-weight) matrices via matmul -> row-collapsed maps [30,512],
then 20 segmented DVE reduces (col_seg is identical on every core, so the
reduce extents are SPMD-safe compile-time constants) -> [30,40] raw sums ->
AllReduce -> scale/bias by host constants -> tiny PE transpose -> template
matmul -> per-row-chunk broadcast matmul -> painted output channels.
"""

import ml_dtypes
import numpy as np

import concourse.bacc as bacc
import concourse.bass as bass
import concourse.mybir as mybir
import concourse.tile as tile
from concourse.bass_utils import run_bass_kernel_spmd

F32 = mybir.dt.float32
F32R = mybir.dt.float32r
BF16 = mybir.dt.bfloat16
AFT = mybir.ActivationFunctionType
AXX = mybir.AxisListType.X

NR, NC_ = 30, 20
H = W = 512
B = 4
CIN = 55
NCORES = 8
DILS = (1, 2, 3)
HALF = 128          # rows per block-diag half (per core: 256 rows = 2x128)
TRP = 16            # row-pairs per x tile
NTILE = HALF // TRP
WPAD = W + 12       # 524


def _build_program(col_bounds):
    """Build + compile the SPMD Bass program. col_bounds: list of (ws, we)
    per col segment (identical across cores -> SPMD-safe constants)."""
    nc = bacc.Bacc(
        "TRN2", target_bir_lowering=False, debug=False, num_devices=NCORES
    )

    xs = nc.dram_tensor("xs", [CIN, 268, WPAD], BF16, kind="ExternalInput").ap()
    wts = nc.dram_tensor("wts", [110, 900], BF16, kind="ExternalInput").ap()
    bias36 = nc.dram_tensor("bias36", [128, 1], F32, kind="ExternalInput").ap()
    r1w = nc.dram_tensor("r1w", [128, 2160], F32, kind="ExternalInput").ap()
    rsel = nc.dram_tensor("rsel", [NR, 256], F32, kind="ExternalInput").ap()
    c1w = nc.dram_tensor("c1w", [NC_, 512], F32, kind="ExternalInput").ap()
    id30 = nc.dram_tensor("id30", [NR, NR], F32, kind="ExternalInput").ap()
    invb = nc.dram_tensor("invb", [NR, 40], F32, kind="ExternalInput").ap()
    bcst = nc.dram_tensor("bcst", [NR, 40], F32, kind="ExternalInput").ap()

    oxs = nc.dram_tensor("oxs", [55, 256, W], F32, kind="ExternalOutput").ap()
    pools = nc.dram_tensor("pools", [NR, NC_], F32, kind="ExternalOutput").ap()

    with tile.TileContext(nc) as tc:
        with (
            tc.tile_pool(name="consts", bufs=1) as consts,
            tc.tile_pool(name="xp", bufs=2) as xp,
            tc.tile_pool(name="featp", bufs=2) as featp,
            tc.tile_pool(name="ftp", bufs=2) as ftp,
            tc.tile_pool(name="misc", bufs=1) as misc,
            tc.tile_pool(name="mapsb", bufs=2) as mapsb,
            tc.tile_pool(name="psconv", bufs=2, space="PSUM") as psconv,
            tc.tile_pool(name="psb", bufs=2, space="PSUM") as psb,
            tc.tile_pool(name="psmap", bufs=2, space="PSUM") as psmap,
            tc.tile_pool(name="dram", bufs=2, space="DRAM") as dram,
        ):
            # ---- constants into SBUF ----
            wt_sb = consts.tile([110, 900], BF16)
            nc.sync.dma_start(wt_sb[:], wts[:])
            bias_sb = consts.tile([128, 1], F32)
            nc.sync.dma_start(bias_sb[:], bias36[:])
            r1w_sb = consts.tile([128, 2160], F32)
            nc.sync.dma_start(r1w_sb[:], r1w[:])
            rsel_sb = consts.tile([NR, 256], F32)
            nc.sync.dma_start(rsel_sb[:], rsel[:])
            c1w_sb = consts.tile([NC_, 512], F32)
            nc.sync.dma_start(c1w_sb[:], c1w[:])
            id30_sb = consts.tile([NR, NR], F32)
            nc.sync.dma_start(id30_sb[:], id30[:])
            inv_sb = consts.tile([NR, 40], F32)
            nc.sync.dma_start(inv_sb[:], invb[:])
            bcst_sb = consts.tile([NR, 40], F32)
            nc.sync.dma_start(bcst_sb[:], bcst[:])

            # ---- phase A: convolutions ----
            for t in range(NTILE):
                xt = xp.tile([110, TRP + 12, WPAD], BF16)
                r0 = TRP * t
                nc.sync.dma_start(xt[0:CIN, :, :], xs[:, r0 : r0 + TRP + 12, :])
                nc.sync.dma_start(
                    xt[CIN : 2 * CIN, :, :],
                    xs[:, HALF + r0 : HALF + r0 + TRP + 12, :],
                )
                fts = [
                    featp.tile([128, 8, 512], F32, tag="feat", name=f"ft{t}_{h}")
                    for h in range(2)
                ]
                for i in range(TRP):
                    ps = psconv.tile([128, 512], F32)
                    for tap in range(25):
                        kh, kw = tap // 5, tap % 5
                        for c in range(3):
                            d = DILS[c]
                            j = i + 6 + d * (kh - 2)
                            cst = 6 + d * (kw - 2)
                            nc.tensor.matmul(
                                ps[32 * c : 32 * c + 12, :],
                                wt_sb[:, (c * 25 + tap) * 12 : (c * 25 + tap) * 12 + 12],
                                xt[:, j, cst : cst + 512],
                                start=(tap == 0),
                                stop=(tap == 24),
                                tile_position=(0, 32 * c),
                            )
                    for c in range(3):
                        ft = fts[i // 8]
                        nc.scalar.activation(
                            ft[32 * c : 32 * c + 12, i % 8, :],
                            ps[32 * c : 32 * c + 12, :],
                            AFT.Relu,
                            bias=bias_sb[32 * c : 32 * c + 12, 0:1],
                        )
                for half in range(2):
                    ft = fts[half]
                    for c in range(3):
                        for ab in range(2):
                            rows0 = ab * HALF + TRP * t + 8 * half
                            nc.sync.dma_start(
                                oxs[36 + c * 6 : 42 + c * 6, rows0 : rows0 + 8, :],
                                ft[32 * c + 6 * ab : 32 * c + 6 * ab + 6, :, :],
                            )

            # ---- phase B: weighted cell sums ----
            ps_wt = psb.tile([NR, 512], F32)
            ps_wb = psb.tile([NR, 512], F32)
            for ch in range(18):
                for q in range(2):
                    ftile = ftp.tile([128, 512], F32)
                    nc.sync.dma_start(
                        ftile[:], oxs[36 + ch, q * 128 : (q + 1) * 128, :]
                    )
                    first = ch == 0 and q == 0
                    last = ch == 17 and q == 1
                    o = (ch * 2 + q) * NR
                    nc.tensor.matmul(
                        ps_wt[:], r1w_sb[:, o : o + NR], ftile[:],
                        start=first, stop=last,
                    )
                    nc.tensor.matmul(
                        ps_wb[:], r1w_sb[:, 1080 + o : 1080 + o + NR], ftile[:],
                        start=first, stop=last,
                    )
            swt = misc.tile([NR, 512], F32)
            nc.scalar.activation(swt[:], ps_wt[:], AFT.Copy)
            swb = misc.tile([NR, 512], F32)
            nc.scalar.activation(swb[:], ps_wb[:], AFT.Copy)
            raw = misc.tile([NR, 40], F32)
            for cb, (ws, we) in enumerate(col_bounds):
                nc.vector.reduce_sum(raw[:, cb : cb + 1], swt[:, ws:we], axis=AXX)
                nc.vector.reduce_sum(
                    raw[:, 20 + cb : 21 + cb], swb[:, ws:we], axis=AXX
                )

            # ---- phase C: all-reduce + stats ----
            cc_in = dram.tile([NR, 40], F32)
            cc_out = dram.tile([NR, 40], F32)
            nc.sync.dma_start(cc_in[:], raw[:])
            nc.gpsimd.collective_compute(
                "AllReduce",
                mybir.AluOpType.add,
                replica_groups=[list(range(NCORES))],
                ins=[cc_in.opt()],
                outs=[cc_out.opt()],
            )
            vals0 = misc.tile([NR, 40], F32)
            nc.sync.dma_start(vals0[:], cc_out[:])
            vals1 = misc.tile([NR, 40], F32)
            nc.vector.tensor_mul(vals1[:], vals0[:], inv_sb[:])
            vals2 = misc.tile([NR, 40], F32)
            nc.vector.tensor_add(vals2[:], vals1[:], bcst_sb[:])

            pool_sb = misc.tile([NR, NC_], F32)
            nc.scalar.activation(pool_sb[:], vals2[:, 20:40], AFT.Sigmoid)
            nc.sync.dma_start(pools[:], pool_sb[:])

            # transpose [30,20] value blocks -> [20,30] lhsT
            ps_tr = psmap.tile([NC_, NR], F32, tag="pm")
            nc.tensor.transpose(ps_tr[:], vals2[:, 0:20], id30_sb[:])
            tmt = misc.tile([NC_, NR], F32)
            nc.scalar.activation(tmt[:], ps_tr[:], AFT.Copy)
            ps_tr2 = psmap.tile([NC_, NR], F32, tag="pm")
            nc.tensor.transpose(ps_tr2[:], vals2[:, 20:40], id30_sb[:])
            bmt = misc.tile([NC_, NR], F32)
            nc.scalar.activation(bmt[:], ps_tr2[:], AFT.Copy)

            # templates [30, 512]
            ps_t1 = psmap.tile([NR, 512], F32, tag="pm")
            nc.tensor.matmul(ps_t1[:], tmt[:], c1w_sb[:], start=True, stop=True)
            ttop = misc.tile([NR, 512], F32)
            nc.scalar.activation(ttop[:], ps_t1[:], AFT.Copy)
            ps_t2 = psmap.tile([NR, 512], F32, tag="pm")
            nc.tensor.matmul(ps_t2[:], bmt[:], c1w_sb[:], start=True, stop=True)
            tsig = misc.tile([NR, 512], F32)
            nc.scalar.activation(tsig[:], ps_t2[:], AFT.Sigmoid)

            # ---- phase D: paint broadcast channels ----
            for q in range(2):
                ps_m = psmap.tile([128, 512], F32, tag="pm")
                nc.tensor.matmul(
                    ps_m[:], rsel_sb[:, q * 128 : (q + 1) * 128], ttop[:],
                    start=True, stop=True,
                )
                mtop = mapsb.tile([128, 512], F32)
                nc.scalar.activation(mtop[:], ps_m[:], AFT.Copy)
                for ch in range(36):
                    nc.sync.dma_start(
                        oxs[ch, q * 128 : (q + 1) * 128, :], mtop[:]
                    )
                ps_m2 = psmap.tile([128, 512], F32, tag="pm")
                nc.tensor.matmul(
                    ps_m2[:], rsel_sb[:, q * 128 : (q + 1) * 128], tsig[:],
                    start=True, stop=True,
                )
                msig = mapsb.tile([128, 512], F32)
                nc.scalar.activation(msig[:], ps_m2[:], AFT.Copy)
                nc.sync.dma_start(oxs[54, q * 128 : (q + 1) * 128, :], msig[:])

    nc.compile()
    return nc


_CACHE = {}


def _get_program(col_seg):
    key = col_seg.tobytes()
    if key not in _CACHE:
        col_bounds = [
            (int(np.searchsorted(col_seg, c, "left")),
             int(np.searchsorted(col_seg, c, "right")))
            for c in range(NC_)
        ]
        _CACHE[key] = _build_program(col_bounds)
    return _CACHE[key]


def kernel(x, row_seg, col_seg, w1, b1, w2, b2, w3, b3, wt, bt, wb, bb):
    x = np.asarray(x, np.float32)
    row_seg = np.asarray(row_seg, np.int32)
    col_seg = np.asarray(col_seg, np.int32)
    ws_ = [np.asarray(w, np.float32) for w in (w1, w2, w3)]
    bs_ = [np.asarray(b, np.float32) for b in (b1, b2, b3)]
    wt = np.asarray(wt, np.float32)
    bt = np.asarray(bt, np.float32)
    wb = np.asarray(wb, np.float32)
    bb = np.asarray(bb, np.float32)

    nc = _get_program(col_seg)

    # ---- host-side constant prep (shared across cores) ----
    wts_np = np.zeros((110, 900), np.float32)
    for c in range(3):
        w = ws_[c]  # [6, 55, 5, 5]
        for tap in range(25):
            kh, kw = tap // 5, tap % 5
            blk = w[:, :, kh, kw].T  # [55, 6]
            col0 = (c * 25 + tap) * 12
            wts_np[0:CIN, col0 : col0 + 6] = blk
            wts_np[CIN : 2 * CIN, col0 + 6 : col0 + 12] = blk

    bias_np = np.zeros((128, 1), np.float32)
    for c in range(3):
        for ab in range(2):
            o = 32 * c + 6 * ab
            bias_np[o : o + 6, 0] = bs_[c]

    wtsum = wt[:, :, 0, 0].sum(axis=0)  # [18]
    wbv = wb[0, :, 0, 0]  # [18]

    c1w_np = np.zeros((NC_, 512), np.float32)
    c1w_np[col_seg, np.arange(W)] = 1.0

    id30_np = np.eye(NR, dtype=np.float32)

    rrows = np.bincount(row_seg, minlength=NR).astype(np.float64)
    ccols = np.bincount(col_seg, minlength=NC_).astype(np.float64)
    cnt = rrows[:, None] * ccols[None, :]  # [30, 20]
    invb_np = np.zeros((NR, 40), np.float32)
    invb_np[:, 0:20] = 1.0 / (cnt * B * 36)
    invb_np[:, 20:40] = 1.0 / (cnt * B * 1)
    bcst_np = np.zeros((NR, 40), np.float32)
    bcst_np[:, 0:20] = bt.mean()
    bcst_np[:, 20:40] = bb[0]

    # ---- per-core inputs ----
    in_maps = []
    for k in range(NCORES):
        b_idx = k // 2
        h0 = 256 * (k % 2)
        xsh = np.zeros((CIN, 268, WPAD), np.float32)
        lo = h0 - 6
        hi = h0 + 262
        slo = max(0, lo)
        shi = min(H, hi)
        xsh[:, slo - lo : shi - lo, 6 : 6 + W] = x[b_idx, :, slo:shi, :]

        hrows = h0 + np.arange(256)
        r1h = np.zeros((2, 128, NR), np.float32)
        for q in range(2):
            r1h[q, np.arange(128), row_seg[hrows[q * 128 : (q + 1) * 128]]] = 1.0
        r1w_np = np.zeros((128, 2160), np.float32)
        for ch in range(18):
            for q in range(2):
                o = (ch * 2 + q) * NR
                r1w_np[:, o : o + NR] = r1h[q] * wtsum[ch]
                r1w_np[:, 1080 + o : 1080 + o + NR] = r1h[q] * wbv[ch]
        rsel_np = np.zeros((NR, 256), np.float32)
        for q in range(2):
            rsel_np[:, q * 128 : (q + 1) * 128] = r1h[q].T

        in_maps.append(
            {
                "xs": xsh.astype(ml_dtypes.bfloat16),
                "wts": wts_np.astype(ml_dtypes.bfloat16),
                "bias36": bias_np,
                "r1w": r1w_np,
                "rsel": rsel_np,
                "c1w": c1w_np,
                "id30": id30_np,
                "invb": invb_np,
                "bcst": bcst_np,
            }
        )

    global _LAST_IN_MAPS
    _LAST_IN_MAPS = in_maps
    res = run_bass_kernel_spmd(nc, in_maps, list(range(NCORES)))

    out_x = np.empty((B, 55, H, W), np.float32)
    for k in range(NCORES):
        b_idx = k // 2
        h0 = 256 * (k % 2)
        out_x[b_idx, :, h0 : h0 + 256, :] = res.results[k]["oxs"]
    pools_np = res.results[0]["pools"].astype(np.float32)
    pools = np.broadcast_to(
        pools_np.reshape(1, 1, NR, NC_), (B, 1, NR, NC_)
    ).copy()
    return out_x, pools
